# revision 33
# baseline (speedup 1.0000x reference)
"""Trainium2 Bass kernel for multi-head attention (B=4, N=1024, C=768, 24 heads x 32).

Sharding: 8 cores = batch(4) x query-half(2). Each core computes K/V for all
1024 keys of its batch but Q for only its 512 queries; outputs are disjoint
[512, 768] slices so the host gather is pure concatenation (no collectives).

Per-core dataflow (all matmuls bf16, fp32 PSUM accumulate):
  QT[768,512], KT[768,1024]  (w stationary, xT moving)
  V [1024, 24*33]            (xT stationary, w_v moving; col 32 of each head
                              slot is 1.0 -> PV matmul also emits softmax denom)
  ST[k,q] per head           row-tiled K=32 matmuls, 4 heads concurrent
  P = exp(ST * scale)        ScalarE, from PSUM, bf16 out
  OT[d,q]+den per head       2x-col-tiled PV matmuls (V' stationary, P moving)
  normalize: recip(den) -> replicate-DMA broadcast -> DVE mul
  y = OTn.T @ wproj          OTn as lhsT; PSUM -> DRAM direct
"""

import os
import threading

import numpy as np
import ml_dtypes

import concourse.bass as bass
import concourse.tile as tile
from bass_rust import add_dep_helper
from concourse import bacc, mybir
from concourse.bass_utils import run_bass_kernel_spmd

bf16 = ml_dtypes.bfloat16
f32 = mybir.dt.float32
bft = mybir.dt.bfloat16

B, N, C = 4, 1024, 768
H, D = 24, 32
NQ = 512  # queries per core
SCALE = float(D) ** -0.5
NCORES = 8

CT = C // 128  # 6 c-tiles
KT = N // 128  # 8 k-tiles
NG = H // 4  # 6 head groups of 4


def build_program(repeat: int | None = None, debug_dump: bool = False):
    nc = bacc.Bacc("TRN2", target_bir_lowering=False, debug=False)

    xT_d = nc.dram_tensor("xT", [C, N], bft, kind="ExternalInput")
    wqkv_d = nc.dram_tensor("wqkv", [C, 3 * C], bft, kind="ExternalInput")
    wproj_d = nc.dram_tensor("wproj", [C, C], bft, kind="ExternalInput")
    y_d = nc.dram_tensor("y", [NQ, C], f32, kind="ExternalOutput")
    global _DEBUG_DUMP
    _DEBUG_DUMP = debug_dump

    with tile.TileContext(nc) as tc:
        if repeat is None:
            with (
                tc.tile_pool(name="sb", bufs=1) as sb,
                tc.tile_pool(name="sb_p", bufs=6) as sb_p,
                tc.tile_pool(name="ps_mm", bufs=2, space="PSUM") as ps_mm,
                tc.tile_pool(name="ps_st", bufs=2, space="PSUM") as ps_st,
                tc.tile_pool(name="ps_pv", bufs=2, space="PSUM") as ps_pv,
            ):
                emit_body(nc, tc, xT_d, wqkv_d, wproj_d, y_d,
                          sb, sb_p, ps_mm, ps_st, ps_pv)
        else:
            with (
                tc.tile_pool(name="sb", bufs=1) as sb,
                tc.tile_pool(name="sb_p", bufs=6) as sb_p,
                tc.tile_pool(name="ps_mm", bufs=2, space="PSUM") as ps_mm,
                tc.tile_pool(name="ps_st", bufs=2, space="PSUM") as ps_st,
                tc.tile_pool(name="ps_pv", bufs=2, space="PSUM") as ps_pv,
            ):
                with tc.For_i(0, repeat, 1):
                    emit_body(nc, tc, xT_d, wqkv_d, wproj_d, y_d,
                              sb, sb_p, ps_mm, ps_st, ps_pv)

    nc.compile()
    return nc


PHASES = {"qkv", "st", "exp", "pv", "norm", "proj", "bcast"}  # ablation hook for perf debugging
_DEBUG_DUMP = False
SERIAL_QKV = False  # emit all QKV projections before attention (debug)


def emit_body(nc, tc, xT_d, wqkv_d, wproj_d, y_d, sb, sb_p, ps_mm, ps_st, ps_pv):
    Exp = mybir.ActivationFunctionType.Exp

    # ---- persistent SBUF tensors ----
    xT = sb.tile([128, CT, N], bft, tag="xT")
    wqkv = sb.tile([128, CT, 3 * C], bft, tag="wqkv")
    wproj = sb.tile([128, CT, C], bft, tag="wproj")
    QT = sb.tile([128, NG, NQ], bft, tag="QT")
    KTs = sb.tile([128, NG, N], bft, tag="KTs")
    # V + 32 replicated ones-columns per head: the PV matmul then emits the
    # softmax denominator pre-broadcast across 32 partitions (rows 32:64 of
    # each head's 64-row block), so no cross-partition broadcast is needed.
    Vp = sb.tile([128, KT, H, 64], bft, tag="Vp")
    OTn = sb.tile([128, NG, NQ], bft, tag="OTn")

    # ---- load inputs ----
    nc.sync.dma_start(xT[:], xT_d[:].rearrange("(t p) n -> p t n", p=128))
    nc.sync.dma_start(wqkv[:], wqkv_d[:].rearrange("(t p) n -> p t n", p=128))
    nc.sync.dma_start(wproj[:], wproj_d[:].rearrange("(t p) n -> p t n", p=128))
    nc.vector.memset(Vp[:, :, :, 32:64], 1.0)

    # ---- QKV projections ----
    def qt_tile(m, after=None):
        ps = ps_mm.tile([128, 512], f32, tag="mm")
        for ct in range(CT):
            for cb in range(2):  # 128x64 col tiling: same PE mode as PV
                mm = nc.tensor.matmul(
                    ps[64 * cb:64 * (cb + 1), :],
                    wqkv[:, ct, m * 128 + 64 * cb:m * 128 + 64 * (cb + 1)],
                    xT[:, ct, 0:NQ],
                    start=(ct == 0), stop=(ct == CT - 1),
                    tile_position=(0, 64 * cb),
                    skip_group_check=True,
                )
                if after is not None:
                    add_dep_helper(mm.ins, after.ins, sync=True,
                                   reason="pin qkv tile after prior group")
        nc.vector.tensor_copy(QT[:, m, :], ps[:])

    def kt_tile(m, after=None):
        for nh in range(2):
            ps = ps_mm.tile([128, 512], f32, tag="mm")
            for ct in range(CT):
                for cb in range(2):
                    mm = nc.tensor.matmul(
                        ps[64 * cb:64 * (cb + 1), :],
                        wqkv[:, ct, C + m * 128 + 64 * cb:C + m * 128 + 64 * (cb + 1)],
                        xT[:, ct, nh * 512:(nh + 1) * 512],
                        start=(ct == 0), stop=(ct == CT - 1),
                        tile_position=(0, 64 * cb),
                        skip_group_check=True,
                    )
                    if after is not None:
                        add_dep_helper(mm.ins, after.ins, sync=True,
                                       reason="pin qkv tile after prior group")
            nc.vector.tensor_copy(KTs[:, m, nh * 512:(nh + 1) * 512], ps[:])

    def v_tile(kt):
        # V k-tile: out [128 keys, 768 cv]; xT stationary, w_v moving
        for (lo, w) in ((0, 512), (512, 256)):
            ps = ps_mm.tile([128, w], f32, tag="mm")
            for ct in range(CT):
                for cb in range(2):
                    nc.tensor.matmul(
                        ps[64 * cb:64 * (cb + 1), :],
                        xT[:, ct, kt * 128 + 64 * cb:kt * 128 + 64 * (cb + 1)],
                        wqkv[:, ct, 2 * C + lo:2 * C + lo + w],
                        start=(ct == 0), stop=(ct == CT - 1),
                        tile_position=(0, 64 * cb),
                        skip_group_check=True,
                    )
            nc.vector.tensor_copy(
                Vp[:, kt, lo // 32:(lo + w) // 32, 0:32],
                ps[:].rearrange("p (h d) -> p h d", d=32),
            )

    # ---- attention for one head group (4 heads) ----
    def attention_group(g, pre_kt=None):
        last_pv = [None]
        pv_ps = [
            ps_pv.tile([128, NQ], f32, tag="pv", name=f"pv_g{g}_{i}")
            for i in range(2)
        ]
        for kt in range(KT):
            if pre_kt is not None:
                pre_kt(kt)
            for half in range(2):
                st = ps_st.tile([128, 1024], f32, tag="st")
                for jj in range(2):
                    if "st" not in PHASES:
                        break
                    j = 2 * half + jj
                    nc.tensor.matmul(
                        st[:, jj * 512:(jj + 1) * 512],
                        KTs[32 * j:32 * (j + 1), g, kt * 128:(kt + 1) * 128],
                        QT[32 * j:32 * (j + 1), g, :],
                        start=True, stop=True,
                        tile_position=(32 * j, 0),
                    )
                p = sb_p.tile([128, 1024], bft, tag="p")
                if "exp" in PHASES:
                    nc.scalar.activation(p[:], st[:], Exp, scale=SCALE)
                for jj in range(2):
                    if "pv" not in PHASES:
                        break
                    h = 4 * g + 2 * half + jj
                    last_pv[0] = nc.tensor.matmul(
                        pv_ps[half][64 * jj:64 * (jj + 1), :],
                        Vp[:, kt, h, :],
                        p[:, jj * 512:(jj + 1) * 512],
                        start=(kt == 0), stop=(kt == KT - 1),
                        tile_position=(0, 64 * jj),
                        # the two col-tiled accumulation groups share a PSUM
                        # bank but write disjoint partitions (0:64 / 64:128)
                        skip_group_check=True,
                    )
        # normalize straight out of PSUM: recip the replicated denominator
        # rows, then one fused multiply per head writing OTn (proj lhsT)
        for half in range(2):
            for jj in range(2):
                j = 2 * half + jj
                dsb = sb_p.tile([32, NQ], f32, tag="dsb",
                                name=f"dsb_g{g}_{half}_{jj}")
                nc.vector.tensor_copy(
                    dsb[:], pv_ps[half][64 * jj + 32:64 * (jj + 1), :])
                recd = sb_p.tile([32, NQ], f32, tag="recd",
                                 name=f"recd_g{g}_{half}_{jj}")
                nc.vector.reciprocal_approx_fast(recd[:], dsb[:])
                nc.vector.tensor_mul(
                    OTn[32 * j:32 * (j + 1), g, :],
                    pv_ps[half][64 * jj:64 * jj + 32, :],
                    recd[:],
                )
                if _DEBUG_DUMP and g == 0 and half == 0 and jj == 0:
                    djunk = sb.tile([32, 2 * NQ], f32, tag="dbg_den0")
                    nc.vector.tensor_copy(
                        djunk[:, 0:NQ], pv_ps[half][32:64, :])
                    nc.vector.tensor_copy(djunk[:, NQ:], recd[:])
                    dj_d = nc.dram_tensor("dbg_den0", [32, 2 * NQ], f32,
                                          kind="ExternalOutput")
                    nc.sync.dma_start(dj_d[:], djunk[:])
        return last_pv[0]

    # ---- emission order: QKV for group 0 + V, then pipeline ----
    if SERIAL_QKV:
        if "qkv" in PHASES:
            for m in range(NG):
                qt_tile(m)
                kt_tile(m)
            for kt in range(KT):
                v_tile(kt)
        for g in range(NG):
            attention_group(g)
    else:
        if "qkv" in PHASES:
            qt_tile(0)
            kt_tile(0)
        for g in range(NG):
            # group 0 interleaves the V projection tiles with its kt loop so
            # the ScalarE exp pipeline starts as early as possible
            lp = attention_group(
                g, pre_kt=v_tile if (g == 0 and "qkv" in PHASES) else None)
            if "qkv" in PHASES and g + 1 < NG:
                qt_tile(g + 1, after=lp)
                kt_tile(g + 1, after=lp)

    if _DEBUG_DUMP:
        for nm, t in [("QT", QT), ("KTs", KTs), ("Vp", Vp), ("OTn", OTn)]:
            dbg = nc.dram_tensor(f"dbg_{nm}", list(t.shape), t.dtype,
                                 kind="ExternalOutput")
            nc.sync.dma_start(dbg[:], t[:])

    # ---- output projection ----
    if "proj" not in PHASES:
        return
    for qt in range(4):
        for (lo, w) in ((0, 512), (512, 256)):
            ps = ps_mm.tile([128, w], f32, tag="mm")
            for m in range(CT):
                for cb in range(2):
                    nc.tensor.matmul(
                        ps[64 * cb:64 * (cb + 1), :],
                        OTn[:, m, qt * 128 + 64 * cb:qt * 128 + 64 * (cb + 1)],
                        wproj[:, m, lo:lo + w],
                        start=(m == 0), stop=(m == CT - 1),
                        tile_position=(0, 64 * cb),
                        skip_group_check=True,
                    )
            ysb = sb_p.tile([128, w], f32, tag=f"y{lo}", name=f"y_{qt}_{lo}")
            nc.vector.tensor_copy(ysb[:], ps[:])
            nc.sync.dma_start(y_d[qt * 128:(qt + 1) * 128, lo:lo + w], ysb[:])



# ------------------------------------------------------------------
# host entry point
# ------------------------------------------------------------------
_NC_LOCK = threading.Lock()
_NC = None
LAST_RESULTS = None


def _get_nc():
    global _NC
    with _NC_LOCK:
        if _NC is None:
            _NC = build_program()
    return _NC


def make_in_maps(x, w_qkv, w_proj):
    xT = np.transpose(np.asarray(x, np.float32), (0, 2, 1))  # [B, C, N]
    wq = np.asarray(w_qkv, np.float32).astype(bf16)
    wp = np.asarray(w_proj, np.float32).astype(bf16)
    in_maps = []
    for core in range(NCORES):
        b, half = divmod(core, 2)
        xt = xT[b]
        if half == 1:
            xt = np.concatenate([xt[:, NQ:], xt[:, :NQ]], axis=1)
        in_maps.append({
            "xT": np.ascontiguousarray(xt).astype(bf16),
            "wqkv": wq,
            "wproj": wp,
        })
    return in_maps


def kernel(x, w_qkv, w_proj):
    global LAST_RESULTS
    nc = _get_nc()
    in_maps = make_in_maps(x, w_qkv, w_proj)
    res = run_bass_kernel_spmd(nc, in_maps, core_ids=list(range(NCORES)))
    LAST_RESULTS = res
    y = np.empty((B, N, C), np.float32)
    for core in range(NCORES):
        b, half = divmod(core, 2)
        y[b, half * NQ:(half + 1) * NQ] = res.results[core]["y"]
    return y


# revision 34
# speedup vs baseline: 1.1774x; 1.1774x over previous
"""Trainium2 Bass kernel for multi-head attention (B=4, N=1024, C=768, 24 heads x 32).

Sharding: 8 cores = batch(4) x query-half(2). Each core computes K/V for all
1024 keys of its batch but Q for only its 512 queries; outputs are disjoint
[512, 768] slices so the host gather is pure concatenation (no collectives).

Per-core dataflow (all matmuls bf16, fp32 PSUM accumulate):
  QT[768,512], KT[768,1024]  (w stationary, xT moving)
  V [1024, 24*33]            (xT stationary, w_v moving; col 32 of each head
                              slot is 1.0 -> PV matmul also emits softmax denom)
  ST[k,q] per head           row-tiled K=32 matmuls, 4 heads concurrent
  P = exp(ST * scale)        ScalarE, from PSUM, bf16 out
  OT[d,q]+den per head       2x-col-tiled PV matmuls (V' stationary, P moving)
  normalize: recip(den) -> replicate-DMA broadcast -> DVE mul
  y = OTn.T @ wproj          OTn as lhsT; PSUM -> DRAM direct
"""

import os
import threading

import numpy as np
import ml_dtypes

import concourse.bass as bass
import concourse.tile as tile
from bass_rust import add_dep_helper
from concourse import bacc, mybir
from concourse.bass_utils import run_bass_kernel_spmd

bf16 = ml_dtypes.bfloat16
f32 = mybir.dt.float32
bft = mybir.dt.bfloat16

B, N, C = 4, 1024, 768
H, D = 24, 32
NQ = 512  # queries per core
SCALE = float(D) ** -0.5
NCORES = 8

CT = C // 128  # 6 c-tiles
KT = N // 128  # 8 k-tiles
NG = H // 4  # 6 head groups of 4


def build_program(repeat: int | None = None, debug_dump: bool = False):
    nc = bacc.Bacc("TRN2", target_bir_lowering=False, debug=False)

    xT_d = nc.dram_tensor("xT", [C, N], bft, kind="ExternalInput")
    wqkv_d = nc.dram_tensor("wqkv", [C, 3 * C], bft, kind="ExternalInput")
    wproj_d = nc.dram_tensor("wproj", [C, C], bft, kind="ExternalInput")
    y_d = nc.dram_tensor("y", [NQ, C], f32, kind="ExternalOutput")
    global _DEBUG_DUMP
    _DEBUG_DUMP = debug_dump

    with tile.TileContext(nc) as tc:
        if repeat is None:
            with (
                tc.tile_pool(name="sb", bufs=1) as sb,
                tc.tile_pool(name="sb_p", bufs=3) as sb_p,
                tc.tile_pool(name="ps_mm", bufs=2, space="PSUM") as ps_mm,
                tc.tile_pool(name="ps_st", bufs=1, space="PSUM") as ps_st,
                tc.tile_pool(name="ps_pv", bufs=2, space="PSUM") as ps_pv,
            ):
                emit_body(nc, tc, xT_d, wqkv_d, wproj_d, y_d,
                          sb, sb_p, ps_mm, ps_st, ps_pv)
        else:
            with (
                tc.tile_pool(name="sb", bufs=1) as sb,
                tc.tile_pool(name="sb_p", bufs=3) as sb_p,
                tc.tile_pool(name="ps_mm", bufs=2, space="PSUM") as ps_mm,
                tc.tile_pool(name="ps_st", bufs=1, space="PSUM") as ps_st,
                tc.tile_pool(name="ps_pv", bufs=2, space="PSUM") as ps_pv,
            ):
                with tc.For_i(0, repeat, 1):
                    emit_body(nc, tc, xT_d, wqkv_d, wproj_d, y_d,
                              sb, sb_p, ps_mm, ps_st, ps_pv)

    nc.compile()
    return nc


PHASES = {"qkv", "st", "exp", "pv", "norm", "proj", "bcast"}  # ablation hook for perf debugging
_DEBUG_DUMP = False
SERIAL_QKV = False  # emit all QKV projections before attention (debug)


def emit_body(nc, tc, xT_d, wqkv_d, wproj_d, y_d, sb, sb_p, ps_mm, ps_st, ps_pv):
    Exp = mybir.ActivationFunctionType.Exp

    # ---- persistent SBUF tensors ----
    xT = sb.tile([128, CT, N], bft, tag="xT")
    wqkv = sb.tile([128, CT, 3 * C], bft, tag="wqkv")
    wproj = sb.tile([128, CT, C], bft, tag="wproj")
    QT = sb.tile([128, NG, NQ], bft, tag="QT")
    KTs = sb.tile([128, NG, N], bft, tag="KTs")
    # V + 32 replicated ones-columns per head: the PV matmul then emits the
    # softmax denominator pre-broadcast across 32 partitions (rows 32:64 of
    # each head's 64-row block), so no cross-partition broadcast is needed.
    Vp = sb.tile([128, KT, H, 64], bft, tag="Vp")
    OTn = sb.tile([128, NG, NQ], bft, tag="OTn")

    # ---- load inputs ----
    xT_v = xT_d[:].rearrange("(t p) n -> p t n", p=128)
    wqkv_v = wqkv_d[:].rearrange("(t p) n -> p t n", p=128)
    wproj_v = wproj_d[:].rearrange("(t p) n -> p t n", p=128)
    for ct in range(CT):
        nc.sync.dma_start(wqkv[:, ct, :], wqkv_v[:, ct, :])
    for ct in range(0, CT, 2):
        nc.sync.dma_start(xT[:, ct:ct + 2, :], xT_v[:, ct:ct + 2, :])
    for ct in range(0, CT, 3):
        nc.sync.dma_start(wproj[:, ct:ct + 3, :], wproj_v[:, ct:ct + 3, :])
    nc.vector.memset(Vp[:, :, :, 32:64], 1.0)

    # ---- QKV projections ----
    def qt_tile(m, after=None):
        ps = ps_mm.tile([128, 512], f32, tag="mm")
        for ct in range(CT):
            for cb in range(2):  # 128x64 col tiling: same PE mode as PV
                mm = nc.tensor.matmul(
                    ps[64 * cb:64 * (cb + 1), :],
                    wqkv[:, ct, m * 128 + 64 * cb:m * 128 + 64 * (cb + 1)],
                    xT[:, ct, 0:NQ],
                    start=(ct == 0), stop=(ct == CT - 1),
                    tile_position=(0, 64 * cb),
                    skip_group_check=True,
                )
                if after is not None:
                    add_dep_helper(mm.ins, after.ins, sync=True,
                                   reason="pin qkv tile after prior group")
        nc.vector.tensor_copy(QT[:, m, :], ps[:])

    def kt_tile(m, after=None):
        for nh in range(2):
            ps = ps_mm.tile([128, 512], f32, tag="mm")
            for ct in range(CT):
                for cb in range(2):
                    mm = nc.tensor.matmul(
                        ps[64 * cb:64 * (cb + 1), :],
                        wqkv[:, ct, C + m * 128 + 64 * cb:C + m * 128 + 64 * (cb + 1)],
                        xT[:, ct, nh * 512:(nh + 1) * 512],
                        start=(ct == 0), stop=(ct == CT - 1),
                        tile_position=(0, 64 * cb),
                        skip_group_check=True,
                    )
                    if after is not None:
                        add_dep_helper(mm.ins, after.ins, sync=True,
                                       reason="pin qkv tile after prior group")
            nc.vector.tensor_copy(KTs[:, m, nh * 512:(nh + 1) * 512], ps[:])

    def v_tile(kt):
        # V k-tile: out [128 keys, 768 cv]; xT stationary, w_v moving
        for (lo, w) in ((0, 512), (512, 256)):
            ps = ps_mm.tile([128, w], f32, tag="mm")
            for ct in range(CT):
                for cb in range(2):
                    nc.tensor.matmul(
                        ps[64 * cb:64 * (cb + 1), :],
                        xT[:, ct, kt * 128 + 64 * cb:kt * 128 + 64 * (cb + 1)],
                        wqkv[:, ct, 2 * C + lo:2 * C + lo + w],
                        start=(ct == 0), stop=(ct == CT - 1),
                        tile_position=(0, 64 * cb),
                        skip_group_check=True,
                    )
            nc.vector.tensor_copy(
                Vp[:, kt, lo // 32:(lo + w) // 32, 0:32],
                ps[:].rearrange("p (h d) -> p h d", d=32),
            )

    # ---- attention for one head group (4 heads) ----
    def attention_group(g, pre_kt=None):
        last_pv = [None]
        pv_ps = [
            ps_pv.tile([128, NQ], f32, tag="pv", name=f"pv_g{g}_{i}")
            for i in range(2)
        ]
        for kt in range(KT):
            if pre_kt is not None:
                pre_kt(kt)
            st = ps_st.tile([128, 2048], f32, tag="st")
            for j in range(4):
                nc.tensor.matmul(
                    st[:, j * 512:(j + 1) * 512],
                    KTs[32 * j:32 * (j + 1), g, kt * 128:(kt + 1) * 128],
                    QT[32 * j:32 * (j + 1), g, :],
                    start=True, stop=True,
                    tile_position=(32 * j, 0),
                )
            p = sb_p.tile([128, 2048], bft, tag="p")
            nc.scalar.activation(p[:], st[:], Exp, scale=SCALE)
            for half in range(2):
                for jj in range(2):
                    j = 2 * half + jj
                    h = 4 * g + j
                    last_pv[0] = nc.tensor.matmul(
                        pv_ps[half][64 * jj:64 * (jj + 1), :],
                        Vp[:, kt, h, :],
                        p[:, j * 512:(j + 1) * 512],
                        start=(kt == 0), stop=(kt == KT - 1),
                        tile_position=(0, 64 * jj),
                        # the two col-tiled accumulation groups share a PSUM
                        # bank but write disjoint partitions (0:64 / 64:128)
                        skip_group_check=True,
                    )
        # normalize: pack the 4 replicated-denominator blocks into one
        # [128, 512] tile, one wide approx reciprocal, 4 fused multiplies
        deng = sb_p.tile([128, NQ], f32, tag="deng", name=f"deng_{g}")
        for half in range(2):
            for jj in range(2):
                j = 2 * half + jj
                nc.vector.tensor_copy(
                    deng[32 * j:32 * (j + 1), :],
                    pv_ps[half][64 * jj + 32:64 * (jj + 1), :])
        recd = sb_p.tile([128, NQ], f32, tag="recd", name=f"recd_{g}")
        nc.vector.reciprocal_approx_fast(recd[:], deng[:])
        for half in range(2):
            for jj in range(2):
                j = 2 * half + jj
                nc.vector.tensor_mul(
                    OTn[32 * j:32 * (j + 1), g, :],
                    pv_ps[half][64 * jj:64 * jj + 32, :],
                    recd[32 * j:32 * (j + 1), :],
                )
        return last_pv[0]

    # ---- emission order: QKV for group 0 + V, then pipeline ----
    if SERIAL_QKV:
        if "qkv" in PHASES:
            for m in range(NG):
                qt_tile(m)
                kt_tile(m)
            for kt in range(KT):
                v_tile(kt)
        for g in range(NG):
            attention_group(g)
    else:
        if "qkv" in PHASES:
            qt_tile(0)
            kt_tile(0)
        for g in range(NG):
            # group 0 interleaves the V projection tiles with its kt loop so
            # the ScalarE exp pipeline starts as early as possible
            lp = attention_group(
                g, pre_kt=v_tile if (g == 0 and "qkv" in PHASES) else None)
            if "qkv" in PHASES and g + 1 < NG:
                qt_tile(g + 1, after=lp)
                kt_tile(g + 1, after=lp)

    if _DEBUG_DUMP:
        for nm, t in [("QT", QT), ("KTs", KTs), ("Vp", Vp), ("OTn", OTn)]:
            dbg = nc.dram_tensor(f"dbg_{nm}", list(t.shape), t.dtype,
                                 kind="ExternalOutput")
            nc.sync.dma_start(dbg[:], t[:])

    # ---- output projection ----
    if "proj" not in PHASES:
        return
    for qt in range(4):
        for (lo, w) in ((0, 512), (512, 256)):
            ps = ps_mm.tile([128, w], f32, tag="mm")
            for m in range(CT):
                for cb in range(2):
                    nc.tensor.matmul(
                        ps[64 * cb:64 * (cb + 1), :],
                        OTn[:, m, qt * 128 + 64 * cb:qt * 128 + 64 * (cb + 1)],
                        wproj[:, m, lo:lo + w],
                        start=(m == 0), stop=(m == CT - 1),
                        tile_position=(0, 64 * cb),
                        skip_group_check=True,
                    )
            ysb = sb_p.tile([128, w], f32, tag=f"y{lo}", name=f"y_{qt}_{lo}")
            nc.vector.tensor_copy(ysb[:], ps[:])
            nc.sync.dma_start(y_d[qt * 128:(qt + 1) * 128, lo:lo + w], ysb[:])



# ------------------------------------------------------------------
# host entry point
# ------------------------------------------------------------------
_NC_LOCK = threading.Lock()
_NC = None
LAST_RESULTS = None


def _get_nc():
    global _NC
    with _NC_LOCK:
        if _NC is None:
            _NC = build_program()
    return _NC


def make_in_maps(x, w_qkv, w_proj):
    xT = np.transpose(np.asarray(x, np.float32), (0, 2, 1))  # [B, C, N]
    wq = np.asarray(w_qkv, np.float32).astype(bf16)
    wp = np.asarray(w_proj, np.float32).astype(bf16)
    in_maps = []
    for core in range(NCORES):
        b, half = divmod(core, 2)
        xt = xT[b]
        if half == 1:
            xt = np.concatenate([xt[:, NQ:], xt[:, :NQ]], axis=1)
        in_maps.append({
            "xT": np.ascontiguousarray(xt).astype(bf16),
            "wqkv": wq,
            "wproj": wp,
        })
    return in_maps


def kernel(x, w_qkv, w_proj):
    global LAST_RESULTS
    nc = _get_nc()
    in_maps = make_in_maps(x, w_qkv, w_proj)
    res = run_bass_kernel_spmd(nc, in_maps, core_ids=list(range(NCORES)))
    LAST_RESULTS = res
    y = np.empty((B, N, C), np.float32)
    for core in range(NCORES):
        b, half = divmod(core, 2)
        y[b, half * NQ:(half + 1) * NQ] = res.results[core]["y"]
    return y


# revision 37
# speedup vs baseline: 1.3949x; 1.1848x over previous
"""Trainium2 Bass kernel for multi-head attention (B=4, N=1024, C=768, 24 heads x 32).

Sharding: 8 cores = batch(4) x query-half(2). Each core computes K/V for all
1024 keys of its batch but Q for only its 512 queries; outputs are disjoint
[512, 768] slices so the host gather is pure concatenation (no collectives).

Per-core dataflow (all matmuls bf16, fp32 PSUM accumulate):
  QT[768,512], KT[768,1024]  (w stationary, xT moving)
  V [1024, 24*33]            (xT stationary, w_v moving; col 32 of each head
                              slot is 1.0 -> PV matmul also emits softmax denom)
  ST[k,q] per head           row-tiled K=32 matmuls, 4 heads concurrent
  P = exp(ST * scale)        ScalarE, from PSUM, bf16 out
  OT[d,q]+den per head       2x-col-tiled PV matmuls (V' stationary, P moving)
  normalize: recip(den) -> replicate-DMA broadcast -> DVE mul
  y = OTn.T @ wproj          OTn as lhsT; PSUM -> DRAM direct
"""

import os
import threading

import numpy as np
import ml_dtypes

import concourse.bass as bass
import concourse.tile as tile
from bass_rust import add_dep_helper
from concourse import bacc, mybir
from concourse.bass_utils import run_bass_kernel_spmd

bf16 = ml_dtypes.bfloat16
f32 = mybir.dt.float32
bft = mybir.dt.bfloat16

B, N, C = 4, 1024, 768
H, D = 24, 32
NQ = 512  # queries per core
SCALE = float(D) ** -0.5
NCORES = 8

CT = C // 128  # 6 c-tiles
KT = N // 128  # 8 k-tiles
NG = H // 4  # 6 head groups of 4


def build_program(repeat: int | None = None, debug_dump: bool = False):
    nc = bacc.Bacc("TRN2", target_bir_lowering=False, debug=False)

    xT_d = nc.dram_tensor("xT", [C, N], bft, kind="ExternalInput")
    wqkv_d = nc.dram_tensor("wqkv", [C, 3 * C], bft, kind="ExternalInput")
    wproj_d = nc.dram_tensor("wproj", [C, C], bft, kind="ExternalInput")
    y_d = nc.dram_tensor("y", [NQ, C], f32, kind="ExternalOutput")
    global _DEBUG_DUMP
    _DEBUG_DUMP = debug_dump

    with tile.TileContext(nc) as tc:
        if repeat is None:
            with (
                tc.tile_pool(name="sb", bufs=1) as sb,
                tc.tile_pool(name="sb_p", bufs=3) as sb_p,
                tc.tile_pool(name="ps_mm", bufs=2, space="PSUM") as ps_mm,
                tc.tile_pool(name="ps_st", bufs=1, space="PSUM") as ps_st,
                tc.tile_pool(name="ps_pv", bufs=2, space="PSUM") as ps_pv,
            ):
                emit_body(nc, tc, xT_d, wqkv_d, wproj_d, y_d,
                          sb, sb_p, ps_mm, ps_st, ps_pv)
        else:
            with (
                tc.tile_pool(name="sb", bufs=1) as sb,
                tc.tile_pool(name="sb_p", bufs=3) as sb_p,
                tc.tile_pool(name="ps_mm", bufs=2, space="PSUM") as ps_mm,
                tc.tile_pool(name="ps_st", bufs=1, space="PSUM") as ps_st,
                tc.tile_pool(name="ps_pv", bufs=2, space="PSUM") as ps_pv,
            ):
                with tc.For_i(0, repeat, 1):
                    emit_body(nc, tc, xT_d, wqkv_d, wproj_d, y_d,
                              sb, sb_p, ps_mm, ps_st, ps_pv)

    nc.compile()
    return nc


PHASES = {"qkv", "st", "exp", "pv", "norm", "proj", "bcast"}  # ablation hook for perf debugging
_DEBUG_DUMP = False
SERIAL_QKV = False  # emit all QKV projections before attention (debug)
ABLATE = set()  # timing ablations: subset of {exp, st, pv, qkv, norm, proj, dma}


def emit_body(nc, tc, xT_d, wqkv_d, wproj_d, y_d, sb, sb_p, ps_mm, ps_st, ps_pv):
    Exp = mybir.ActivationFunctionType.Exp

    # ---- persistent SBUF tensors ----
    xT = sb.tile([128, CT, N], bft, tag="xT")
    wqkv = sb.tile([128, CT, 3 * C], bft, tag="wqkv")
    wproj = sb.tile([128, CT, C], bft, tag="wproj")
    QT = sb.tile([128, NG, NQ], bft, tag="QT")
    KTs = sb.tile([128, NG, N], bft, tag="KTs")
    # V + 32 replicated ones-columns per head: the PV matmul then emits the
    # softmax denominator pre-broadcast across 32 partitions (rows 32:64 of
    # each head's 64-row block), so no cross-partition broadcast is needed.
    Vp = sb.tile([128, KT, H, 64], bft, tag="Vp")
    OTn = sb.tile([128, NG, NQ], bft, tag="OTn")

    # ---- load inputs ----
    xT_v = xT_d[:].rearrange("(t p) n -> p t n", p=128)
    wqkv_v = wqkv_d[:].rearrange("(t p) n -> p t n", p=128)
    wproj_v = wproj_d[:].rearrange("(t p) n -> p t n", p=128)
    for ct in range(CT):
        nc.sync.dma_start(wqkv[:, ct, :], wqkv_v[:, 0 if 'dma' in ABLATE else ct, :])
    for ct in range(0, CT, 2):
        src_ct = 0 if 'dma' in ABLATE else ct
        nc.sync.dma_start(xT[:, ct:ct + 2, :], xT_v[:, src_ct:src_ct + 2, :])
    for ct in range(0, CT, 3):
        src_ct = 0 if 'dma' in ABLATE else ct
        nc.sync.dma_start(wproj[:, ct:ct + 3, :], wproj_v[:, src_ct:src_ct + 3, :])
    nc.vector.memset(Vp[:, :, :, 32:64], 1.0)

    # ---- QKV projections ----
    def qt_tile(m, after=None):
        if "qkv" in ABLATE:
            nc.vector.tensor_copy(QT[0:1, m, 0:8], xT[0:1, 0, 0:8])
            return
        ps = ps_mm.tile([128, 512], f32, tag="mm")
        for ct in range(CT):
            mm = nc.tensor.matmul(
                ps[:],
                wqkv[:, ct, m * 128:(m + 1) * 128],
                xT[:, ct, 0:NQ],
                start=(ct == 0), stop=(ct == CT - 1),
            )
            if after is not None:
                add_dep_helper(mm.ins, after.ins, sync=True,
                               reason="pin qkv tile after prior group")
        nc.vector.tensor_copy(QT[:, m, :], ps[:])

    def kt_tile(m, after=None):
        if "qkv" in ABLATE:
            nc.vector.tensor_copy(KTs[0:1, m, 0:8], xT[0:1, 0, 0:8])
            return
        for nh in range(2):
            ps = ps_mm.tile([128, 512], f32, tag="mm")
            for ct in range(CT):
                mm = nc.tensor.matmul(
                    ps[:],
                    wqkv[:, ct, C + m * 128:C + (m + 1) * 128],
                    xT[:, ct, nh * 512:(nh + 1) * 512],
                    start=(ct == 0), stop=(ct == CT - 1),
                )
                if after is not None:
                    add_dep_helper(mm.ins, after.ins, sync=True,
                                   reason="pin qkv tile after prior group")
            nc.vector.tensor_copy(KTs[:, m, nh * 512:(nh + 1) * 512], ps[:])

    def v_tile(kt):
        if "qkv" in ABLATE:
            nc.vector.tensor_copy(Vp[0:1, kt, 0, 0:8], xT[0:1, 0, 0:8])
            return
        # V k-tile: out [128 keys, 768 cv]; xT stationary, w_v moving
        for (lo, w) in ((0, 512), (512, 256)):
            ps = ps_mm.tile([128, w], f32, tag="mm")
            for ct in range(CT):
                nc.tensor.matmul(
                    ps[:],
                    xT[:, ct, kt * 128:(kt + 1) * 128],
                    wqkv[:, ct, 2 * C + lo:2 * C + lo + w],
                    start=(ct == 0), stop=(ct == CT - 1),
                )
            nc.vector.tensor_copy(
                Vp[:, kt, lo // 32:(lo + w) // 32, 0:32],
                ps[:].rearrange("p (h d) -> p h d", d=32),
            )

    # ---- attention for one head group (4 heads) ----
    def attention_group(g, pre_kt=None):
        last_pv = [None]
        pv_ps = [
            ps_pv.tile([128, NQ], f32, tag="pv", name=f"pv_g{g}_{i}")
            for i in range(2)
        ]
        for kt in range(KT):
            if pre_kt is not None:
                pre_kt(kt)
            st = ps_st.tile([128, 2048], f32, tag="st")
            if "st" in ABLATE:
                nc.vector.tensor_copy(st[0:1, 0:8], QT[0:1, g, 0:8])
            else:
                for j in range(4):
                    nc.tensor.matmul(
                        st[:, j * 512:(j + 1) * 512],
                        KTs[32 * j:32 * (j + 1), g, kt * 128:(kt + 1) * 128],
                        QT[32 * j:32 * (j + 1), g, :],
                        start=True, stop=True,
                        tile_position=(32 * j, 0),
                    )
            p = sb_p.tile([128, 2048], bft, tag="p")
            if "exp" in ABLATE:
                nc.vector.tensor_copy(p[0:1, 0:8], st[0:1, 0:8])
            else:
                nc.scalar.activation(p[:], st[:], Exp, scale=SCALE)
            for half in range(2):
                if "pv" in ABLATE:
                    cp = nc.vector.tensor_copy(
                        pv_ps[half][0:1, 8 * kt:8 * kt + 8], p[0:1, 0:8])
                    last_pv[0] = cp
                    continue
                for jj in range(2):
                    j = 2 * half + jj
                    h = 4 * g + j
                    last_pv[0] = nc.tensor.matmul(
                        pv_ps[half][64 * jj:64 * (jj + 1), :],
                        Vp[:, kt, h, :],
                        p[:, j * 512:(j + 1) * 512],
                        start=(kt == 0), stop=(kt == KT - 1),
                        tile_position=(0, 64 * jj),
                        # the two col-tiled accumulation groups share a PSUM
                        # bank but write disjoint partitions (0:64 / 64:128)
                        skip_group_check=True,
                    )
        # normalize: pack the 4 replicated-denominator blocks into one
        # [128, 512] tile, one wide approx reciprocal, 4 fused multiplies
        if "norm" in ABLATE:
            nc.vector.tensor_copy(OTn[0:1, g, 0:8], pv_ps[0][0:1, 0:8])
            nc.vector.tensor_copy(OTn[1:2, g, 0:8], pv_ps[1][0:1, 0:8])
            return last_pv[0]
        deng = sb_p.tile([128, NQ], f32, tag="deng", name=f"deng_{g}")
        for half in range(2):
            for jj in range(2):
                j = 2 * half + jj
                nc.vector.tensor_copy(
                    deng[32 * j:32 * (j + 1), :],
                    pv_ps[half][64 * jj + 32:64 * (jj + 1), :])
        recd = sb_p.tile([128, NQ], f32, tag="recd", name=f"recd_{g}")
        nc.vector.reciprocal_approx_fast(recd[:], deng[:])
        for half in range(2):
            for jj in range(2):
                j = 2 * half + jj
                nc.vector.tensor_mul(
                    OTn[32 * j:32 * (j + 1), g, :],
                    pv_ps[half][64 * jj:64 * jj + 32, :],
                    recd[32 * j:32 * (j + 1), :],
                )
        return last_pv[0]

    # ---- emission order: QKV for group 0 + V, then pipeline ----
    if SERIAL_QKV:
        if "qkv" in PHASES:
            for m in range(NG):
                qt_tile(m)
                kt_tile(m)
            for kt in range(KT):
                v_tile(kt)
        for g in range(NG):
            attention_group(g)
    else:
        if "qkv" in PHASES:
            qt_tile(0)
            kt_tile(0)
        for g in range(NG):
            # group 0 interleaves the V projection tiles with its kt loop so
            # the ScalarE exp pipeline starts as early as possible
            lp = attention_group(
                g, pre_kt=v_tile if (g == 0 and "qkv" in PHASES) else None)
            if "qkv" in PHASES and g + 1 < NG:
                qt_tile(g + 1, after=lp)
                kt_tile(g + 1, after=lp)

    if _DEBUG_DUMP:
        for nm, t in [("QT", QT), ("KTs", KTs), ("Vp", Vp), ("OTn", OTn)]:
            dbg = nc.dram_tensor(f"dbg_{nm}", list(t.shape), t.dtype,
                                 kind="ExternalOutput")
            nc.sync.dma_start(dbg[:], t[:])

    # ---- output projection ----
    if "proj" in ABLATE:
        ysb0 = sb_p.tile([1, 8], f32, tag="yab")
        nc.vector.tensor_copy(ysb0[:], OTn[0:1, 0, 0:8])
        nc.sync.dma_start(y_d[0:1, 0:8], ysb0[:])
        return
    if "proj" not in PHASES:
        return
    for qt in range(4):
        for (lo, w) in ((0, 512), (512, 256)):
            ps = ps_mm.tile([128, w], f32, tag="mm")
            for m in range(CT):
                nc.tensor.matmul(
                    ps[:],
                    OTn[:, m, qt * 128:(qt + 1) * 128],
                    wproj[:, m, lo:lo + w],
                    start=(m == 0), stop=(m == CT - 1),
                )
            ysb = sb_p.tile([128, w], f32, tag=f"y{lo}", name=f"y_{qt}_{lo}")
            nc.vector.tensor_copy(ysb[:], ps[:])
            nc.sync.dma_start(y_d[qt * 128:(qt + 1) * 128, lo:lo + w], ysb[:])



# ------------------------------------------------------------------
# host entry point
# ------------------------------------------------------------------
_NC_LOCK = threading.Lock()
_NC = None
LAST_RESULTS = None


def _get_nc():
    global _NC
    with _NC_LOCK:
        if _NC is None:
            _NC = build_program()
    return _NC


def make_in_maps(x, w_qkv, w_proj):
    xT = np.transpose(np.asarray(x, np.float32), (0, 2, 1))  # [B, C, N]
    wq = np.asarray(w_qkv, np.float32).astype(bf16)
    wp = np.asarray(w_proj, np.float32).astype(bf16)
    in_maps = []
    for core in range(NCORES):
        b, half = divmod(core, 2)
        xt = xT[b]
        if half == 1:
            xt = np.concatenate([xt[:, NQ:], xt[:, :NQ]], axis=1)
        in_maps.append({
            "xT": np.ascontiguousarray(xt).astype(bf16),
            "wqkv": wq,
            "wproj": wp,
        })
    return in_maps


def kernel(x, w_qkv, w_proj):
    global LAST_RESULTS
    nc = _get_nc()
    in_maps = make_in_maps(x, w_qkv, w_proj)
    res = run_bass_kernel_spmd(nc, in_maps, core_ids=list(range(NCORES)))
    LAST_RESULTS = res
    y = np.empty((B, N, C), np.float32)
    for core in range(NCORES):
        b, half = divmod(core, 2)
        y[b, half * NQ:(half + 1) * NQ] = res.results[core]["y"]
    return y


# revision 43
# speedup vs baseline: 1.9534x; 1.4003x over previous
"""Trainium2 Bass kernel for multi-head attention (B=4, N=1024, C=768, 24 heads x 32).

Sharding: 8 cores = batch(4) x query-half(2). Each core computes K/V for all
1024 keys of its batch but Q for only its 512 queries; outputs are disjoint
[512, 768] slices so the host gather is pure concatenation (no collectives).

Per-core dataflow (all matmuls bf16, fp32 PSUM accumulate):
  QT[768,512], KT[768,1024]  (w stationary, xT moving)
  V [1024, 24*33]            (xT stationary, w_v moving; col 32 of each head
                              slot is 1.0 -> PV matmul also emits softmax denom)
  ST[k,q] per head           row-tiled K=32 matmuls, 4 heads concurrent
  P = exp(ST * scale)        ScalarE, from PSUM, bf16 out
  OT[d,q]+den per head       2x-col-tiled PV matmuls (V' stationary, P moving)
  normalize: recip(den) -> replicate-DMA broadcast -> DVE mul
  y = OTn.T @ wproj          OTn as lhsT; PSUM -> DRAM direct
"""

import os
import threading

import numpy as np
import ml_dtypes

import concourse.bass as bass
import concourse.tile as tile
from bass_rust import add_dep_helper
from concourse import bacc, mybir
from concourse.bass_utils import run_bass_kernel_spmd

bf16 = ml_dtypes.bfloat16
f32 = mybir.dt.float32
bft = mybir.dt.bfloat16

B, N, C = 4, 1024, 768
H, D = 24, 32
NQ = 512  # queries per core
SCALE = float(D) ** -0.5
NCORES = 8

CT = C // 128  # 6 c-tiles
KT = N // 128  # 8 k-tiles
NG = H // 4  # 6 head groups of 4


def build_program(repeat: int | None = None, debug_dump: bool = False):
    nc = bacc.Bacc("TRN2", target_bir_lowering=False, debug=False)

    xT_d = nc.dram_tensor("xT", [C, N], bft, kind="ExternalInput")
    wqkv_d = nc.dram_tensor("wqkv", [C, 3 * C], bft, kind="ExternalInput")
    wproj_d = nc.dram_tensor("wproj", [C, C], bft, kind="ExternalInput")
    y_d = nc.dram_tensor("y", [NQ, C], f32, kind="ExternalOutput")
    global _DEBUG_DUMP
    _DEBUG_DUMP = debug_dump

    with tile.TileContext(nc) as tc:
        if repeat is None:
            with (
                tc.tile_pool(name="sb", bufs=1) as sb,
                tc.tile_pool(name="sb_p", bufs=4) as sb_p,
                tc.tile_pool(name="ps_mm", bufs=2, space="PSUM") as ps_mm,
                tc.tile_pool(name="ps_st", bufs=1, space="PSUM") as ps_st,
                tc.tile_pool(name="ps_pv", bufs=2, space="PSUM") as ps_pv,
            ):
                emit_body(nc, tc, xT_d, wqkv_d, wproj_d, y_d,
                          sb, sb_p, ps_mm, ps_st, ps_pv)
        else:
            with (
                tc.tile_pool(name="sb", bufs=1) as sb,
                tc.tile_pool(name="sb_p", bufs=4) as sb_p,
                tc.tile_pool(name="ps_mm", bufs=2, space="PSUM") as ps_mm,
                tc.tile_pool(name="ps_st", bufs=1, space="PSUM") as ps_st,
                tc.tile_pool(name="ps_pv", bufs=2, space="PSUM") as ps_pv,
            ):
                with tc.For_i(0, repeat, 1):
                    emit_body(nc, tc, xT_d, wqkv_d, wproj_d, y_d,
                              sb, sb_p, ps_mm, ps_st, ps_pv)

    nc.compile()
    return nc


PHASES = {"qkv", "st", "exp", "pv", "norm", "proj", "bcast"}  # ablation hook for perf debugging
_DEBUG_DUMP = False
SERIAL_QKV = False  # emit all QKV projections before attention (debug)
ABLATE = set()  # timing ablations: subset of {exp, st, pv, qkv, norm, proj, dma}


def emit_body(nc, tc, xT_d, wqkv_d, wproj_d, y_d, sb, sb_p, ps_mm, ps_st, ps_pv):
    Exp = mybir.ActivationFunctionType.Exp

    # ---- persistent SBUF tensors ----
    xT = sb.tile([128, CT, N], bft, tag="xT")
    wqkv = sb.tile([128, CT, 3 * C], bft, tag="wqkv")
    wproj = sb.tile([128, CT, C], bft, tag="wproj")
    QT = sb.tile([128, NG, NQ], bft, tag="QT")
    KTs = sb.tile([128, NG, N], bft, tag="KTs")
    # V + 32 replicated ones-columns per head: the PV matmul then emits the
    # softmax denominator pre-broadcast across 32 partitions (rows 32:64 of
    # each head's 64-row block), so no cross-partition broadcast is needed.
    Vp = sb.tile([128, KT, H, 64], bft, tag="Vp")
    OTn = sb.tile([128, NG, NQ], bft, tag="OTn")

    # ---- load inputs ----
    xT_v = xT_d[:].rearrange("(t p) n -> p t n", p=128)
    wqkv_v = wqkv_d[:].rearrange("(t p) n -> p t n", p=128)
    wproj_v = wproj_d[:].rearrange("(t p) n -> p t n", p=128)
    for ct in range(CT):
        nc.sync.dma_start(wqkv[:, ct, :], wqkv_v[:, 0 if 'dma' in ABLATE else ct, :])
    for ct in range(0, CT, 2):
        src_ct = 0 if 'dma' in ABLATE else ct
        nc.sync.dma_start(xT[:, ct:ct + 2, :], xT_v[:, src_ct:src_ct + 2, :])
    for ct in range(0, CT, 3):
        src_ct = 0 if 'dma' in ABLATE else ct
        nc.sync.dma_start(wproj[:, ct:ct + 3, :], wproj_v[:, src_ct:src_ct + 3, :])
    nc.vector.memset(Vp[:, :, :, 32:64], 1.0)

    # ---- QKV projections ----
    def qt_tile(m, after=None):
        if "qkv" in ABLATE:
            nc.vector.tensor_copy(QT[0:1, m, 0:8], xT[0:1, 0, 0:8])
            return
        ps = ps_mm.tile([128, 512], f32, tag="mm")
        for ct in range(CT):
            mm = nc.tensor.matmul(
                ps[:],
                wqkv[:, ct, m * 128:(m + 1) * 128],
                xT[:, ct, 0:NQ],
                start=(ct == 0), stop=(ct == CT - 1),
            )
            if after is not None:
                add_dep_helper(mm.ins, after.ins, sync=True,
                               reason="pin qkv tile after prior group")
        nc.vector.tensor_copy(QT[:, m, :], ps[:])

    def kt_tile(m, after=None, only_nh=None):
        if "qkv" in ABLATE:
            nc.vector.tensor_copy(KTs[0:1, m, 0:8], xT[0:1, 0, 0:8])
            return
        for nh in range(2):
            if only_nh is not None and nh != only_nh:
                continue
            ps = ps_mm.tile([128, 512], f32, tag="mm")
            for ct in range(CT):
                mm = nc.tensor.matmul(
                    ps[:],
                    wqkv[:, ct, C + m * 128:C + (m + 1) * 128],
                    xT[:, ct, nh * 512:(nh + 1) * 512],
                    start=(ct == 0), stop=(ct == CT - 1),
                )
                if after is not None:
                    add_dep_helper(mm.ins, after.ins, sync=True,
                                   reason="pin qkv tile after prior group")
            nc.vector.tensor_copy(KTs[:, m, nh * 512:(nh + 1) * 512], ps[:])

    def v_tile(kt):
        if "qkv" in ABLATE:
            nc.vector.tensor_copy(Vp[0:1, kt, 0, 0:8], xT[0:1, 0, 0:8])
            return
        # V k-tile: out [128 keys, 768 cv]; xT stationary, w_v moving
        for (lo, w) in ((0, 512), (512, 256)):
            ps = ps_mm.tile([128, w], f32, tag="mm")
            for ct in range(CT):
                nc.tensor.matmul(
                    ps[:],
                    xT[:, ct, kt * 128:(kt + 1) * 128],
                    wqkv[:, ct, 2 * C + lo:2 * C + lo + w],
                    start=(ct == 0), stop=(ct == CT - 1),
                )
            nc.vector.tensor_copy(
                Vp[:, kt, lo // 32:(lo + w) // 32, 0:32],
                ps[:].rearrange("p (h d) -> p h d", d=32),
            )

    # ---- attention for one head group (4 heads) ----
    def attention_group(g, pre_kt=None):
        last_pv = [None]
        pv_ps = [
            ps_pv.tile([128, NQ], f32, tag="pv", name=f"pv_g{g}_{i}")
            for i in range(2)
        ]
        for kt in range(KT):
            if pre_kt is not None:
                pre_kt(kt)
            st = ps_st.tile([128, 2048], f32, tag="st")
            if "st" in ABLATE:
                nc.vector.tensor_copy(st[0:1, 0:8], QT[0:1, g, 0:8])
            else:
                for j in range(4):
                    nc.tensor.matmul(
                        st[:, j * 512:(j + 1) * 512],
                        KTs[32 * j:32 * (j + 1), g, kt * 128:(kt + 1) * 128],
                        QT[32 * j:32 * (j + 1), g, :],
                        start=True, stop=True,
                        tile_position=(32 * j, 0),
                    )
            p = sb_p.tile([128, 2048], bft, tag="p")
            if "exp" in ABLATE:
                nc.vector.tensor_copy(p[0:1, 0:8], st[0:1, 0:8])
            else:
                nc.scalar.activation(p[:], st[:], Exp, scale=SCALE)
            for half in range(2):
                if "pv" in ABLATE:
                    cp = nc.vector.tensor_copy(
                        pv_ps[half][0:1, 8 * kt:8 * kt + 8], p[0:1, 0:8])
                    last_pv[0] = cp
                    continue
                for jj in range(2):
                    j = 2 * half + jj
                    h = 4 * g + j
                    last_pv[0] = nc.tensor.matmul(
                        pv_ps[half][64 * jj:64 * (jj + 1), :],
                        Vp[:, kt, h, :],
                        p[:, j * 512:(j + 1) * 512],
                        start=(kt == 0), stop=(kt == KT - 1),
                        tile_position=(0, 64 * jj),
                        # the two col-tiled accumulation groups share a PSUM
                        # bank but write disjoint partitions (0:64 / 64:128)
                        skip_group_check=True,
                    )
        # normalize: pack the 4 replicated-denominator blocks into one
        # [128, 512] tile, one wide approx reciprocal, 4 fused multiplies
        if "norm" in ABLATE:
            nc.vector.tensor_copy(OTn[0:1, g, 0:8], pv_ps[0][0:1, 0:8])
            nc.vector.tensor_copy(OTn[1:2, g, 0:8], pv_ps[1][0:1, 0:8])
            return last_pv[0]
        deng = sb_p.tile([128, NQ], f32, tag="deng", name=f"deng_{g}")
        for half in range(2):
            for jj in range(2):
                j = 2 * half + jj
                nc.vector.tensor_copy(
                    deng[32 * j:32 * (j + 1), :],
                    pv_ps[half][64 * jj + 32:64 * (jj + 1), :])
        recd = sb_p.tile([128, NQ], f32, tag="recd", name=f"recd_{g}")
        nc.vector.reciprocal_approx_fast(recd[:], deng[:])
        for half in range(2):
            for jj in range(2):
                j = 2 * half + jj
                nc.vector.tensor_mul(
                    OTn[32 * j:32 * (j + 1), g, :],
                    pv_ps[half][64 * jj:64 * jj + 32, :],
                    recd[32 * j:32 * (j + 1), :],
                )
        return last_pv[0]

    # ---- emission order: QKV for group 0 + V, then pipeline ----
    if SERIAL_QKV:
        if "qkv" in PHASES:
            for m in range(NG):
                qt_tile(m)
                kt_tile(m)
            for kt in range(KT):
                v_tile(kt)
        for g in range(NG):
            attention_group(g)
    else:
        if "qkv" in PHASES:
            qt_tile(0)
            kt_tile(0)

        def make_pre(g):
            # spread next group's projections across this group's kt loop so
            # the PE fills ScalarE exp windows instead of bursting at the end
            if "qkv" not in PHASES:
                return None
            if g == 0:
                def pre(kt):
                    v_tile(kt)
                    if kt == 5:
                        qt_tile(1)
                    elif kt == 6:
                        kt_tile(1, only_nh=0)
                    elif kt == 7:
                        kt_tile(1, only_nh=1)
                return pre
            return None

        # simpler: group g (>=1) emits group g+1's tiles at kts 3/5/7
        for g in range(NG):
            if g == 0:
                lp = attention_group(g, pre_kt=make_pre(0))
            else:
                nxt = g + 1

                def pre(kt, nxt=nxt):
                    if nxt >= NG or "qkv" not in PHASES:
                        return
                    if kt == 3:
                        qt_tile(nxt)
                    elif kt == 5:
                        kt_tile(nxt, only_nh=0)
                    elif kt == 7:
                        kt_tile(nxt, only_nh=1)
                lp = attention_group(g, pre_kt=pre)

    if _DEBUG_DUMP:
        for nm, t in [("QT", QT), ("KTs", KTs), ("Vp", Vp), ("OTn", OTn)]:
            dbg = nc.dram_tensor(f"dbg_{nm}", list(t.shape), t.dtype,
                                 kind="ExternalOutput")
            nc.sync.dma_start(dbg[:], t[:])

    # ---- output projection ----
    if "proj" in ABLATE:
        ysb0 = sb_p.tile([1, 8], f32, tag="yab")
        nc.vector.tensor_copy(ysb0[:], OTn[0:1, 0, 0:8])
        nc.sync.dma_start(y_d[0:1, 0:8], ysb0[:])
        return
    if "proj" not in PHASES:
        return
    for qt in range(4):
        for (lo, w) in ((0, 512), (512, 256)):
            ps = ps_mm.tile([128, w], f32, tag="mm")
            for m in range(CT):
                nc.tensor.matmul(
                    ps[:],
                    OTn[:, m, qt * 128:(qt + 1) * 128],
                    wproj[:, m, lo:lo + w],
                    start=(m == 0), stop=(m == CT - 1),
                )
            ysb = sb_p.tile([128, w], f32, tag=f"y{lo}", name=f"y_{qt}_{lo}")
            nc.vector.tensor_copy(ysb[:], ps[:])
            nc.sync.dma_start(y_d[qt * 128:(qt + 1) * 128, lo:lo + w], ysb[:])



# ------------------------------------------------------------------
# host entry point
# ------------------------------------------------------------------
_NC_LOCK = threading.Lock()
_NC = None
LAST_RESULTS = None


def _get_nc():
    global _NC
    with _NC_LOCK:
        if _NC is None:
            _NC = build_program()
    return _NC


def make_in_maps(x, w_qkv, w_proj):
    xT = np.transpose(np.asarray(x, np.float32), (0, 2, 1))  # [B, C, N]
    wq = np.asarray(w_qkv, np.float32).astype(bf16)
    wp = np.asarray(w_proj, np.float32).astype(bf16)
    in_maps = []
    for core in range(NCORES):
        b, half = divmod(core, 2)
        xt = xT[b]
        if half == 1:
            xt = np.concatenate([xt[:, NQ:], xt[:, :NQ]], axis=1)
        in_maps.append({
            "xT": np.ascontiguousarray(xt).astype(bf16),
            "wqkv": wq,
            "wproj": wp,
        })
    return in_maps


def kernel(x, w_qkv, w_proj):
    global LAST_RESULTS
    nc = _get_nc()
    in_maps = make_in_maps(x, w_qkv, w_proj)
    res = run_bass_kernel_spmd(nc, in_maps, core_ids=list(range(NCORES)))
    LAST_RESULTS = res
    y = np.empty((B, N, C), np.float32)
    for core in range(NCORES):
        b, half = divmod(core, 2)
        y[b, half * NQ:(half + 1) * NQ] = res.results[core]["y"]
    return y
